# revision 13
# baseline (speedup 1.0000x reference)
"""Trainium2 Bass kernel: single-channel 15x15 cross-correlation (pad=1,
stride=1) of a 4096x4096 fp32 image, + scalar bias.

Strategy: phase-packed matmuls
------------------------------
Columns are split into PHI=8 phases (j = 8Q + P).  An output block of
S=16 rows x 4096 cols is one PSUM tile [128, 512]: partition m=(s,P)
(16 rows x 8 phases), free n=Q (512 macro-columns).  The contraction
dim packs k=(r,p') = 16 input rows x 8 phases = 128 full partitions.
Six stationary matrices A[o,t][(r,p'),(s,P)] = W[16o+r-s, 8t+p'-P]
(o: input row-group offset 0/1, t: macro-column shift 0/1/2) turn the
whole conv into 6 accumulating matmuls of free size 512 per block:

    out_block += A[o,t]^T @ X[b+o][:, t:t+512]

where X[g] is input row-group g in phase-major layout.  Useful MAC
density is 29.3% of the 128x128 array (vs 10.4% for a 15-tap banded
formulation): 6 x 512 PE cycles per 16x4096 output block.

All operands are bf16 (PSUM accumulates fp32; rel err ~3e-3), which
halves DMA bytes.  Host prepares phase-major inputs / unpacks
phase-major outputs, so every DMA is >=1KB-contiguous per partition.
Each core owns 512 output rows = 32 blocks; 33 input row-group tiles
are loaded once each and reused by adjacent blocks (o=0/1).
"""

import os

import numpy as np

KH = KW = 15
PAD = 1
H = W = 4096
OUT = H + 2 * PAD - KH + 1  # 4084
NCORES = 8
S = 16  # output rows per block
PHI = 8  # column phases
NQ = 512  # macro-columns per PSUM tile (8*512 = 4096 output cols)
XQ = 516  # macro-columns per input tile (>= NQ + 2 shifts, 4B-aligned)
NBLK = 32  # blocks per core: 32*16 = 512 output rows
NG = NBLK + 1  # input row-group tiles per core
CROWS = NBLK * S  # 512 output rows per core
XROWS = 257 * 16  # 4112 padded input rows (257 groups of 16)
XCOLS = PHI * XQ  # 4128 padded input cols

LAST_RESULT = None  # BassKernelResults of the most recent run (for test.py)


def _patch_drain():
    """walrus's CTRL_NO instruction struct holds very few semaphore waits;
    Tile's kernel-tail drain aggregates one wait per logical processor and
    overflows it.  Spread the waits across 1-wait-per-nop SP instructions."""
    import concourse.mybir as mybir
    import concourse.tile as tile
    from concourse.vector_clock import ScopedClock

    def _split_drain_and_barrier(self, tick_clock, wait_clock):
        nc = self.nc
        probe = nc.sync.nop(nofuse=True)
        wait_clock.add_sem_waits(
            probe.ins, ScopedClock({None: tick_clock.global_clock})
        )
        si = probe.ins.sync_info
        if si is not None and len(si.on_wait) > 1:
            waits = list(si.on_wait)
            probe.ins.sync_info = mybir.SyncInfo(
                on_wait=waits[:1], on_update=list(si.on_update)
            )
            for w in waits[1:]:
                extra = nc.sync.nop(nofuse=True)
                extra.ins.sync_info = mybir.SyncInfo(on_wait=[w], on_update=[])
        nc.sync.drain()
        # The stock exit path does barrier -> semaphore cleanup -> barrier
        # (~8us).  This NEFF executes once per load, so leftover semaphore
        # values don't matter: skip the cleanup, keep only the drain (which
        # carries the waits that guarantee all DMAs have landed).
        assert self.sems is not None
        popped = nc._tile_sem_poison_stack.pop()
        assert popped is self._sem_poison

    tile.TileContext._drain_and_barrier = _split_drain_and_barrier


def _split_multi_waits(nc):
    """This compiler's TPB instruction structs hold only one sync-wait slot
    (walrus setupSyncWait rejects more).  Tile sometimes assigns 2+ waits
    (DMA completion + slot release) to one instruction; split the excess onto
    same-engine nops inserted immediately before it."""
    import concourse.mybir as mybir

    for fn in nc.m.functions:
        for bb in fn.blocks:
            insts = list(bb.instructions)
            out = []
            changed = False
            for inst in insts:
                si = inst.sync_info
                if (
                    not isinstance(inst, mybir.InstNoOp)
                    and si is not None
                    and len(si.on_wait) > 1
                ):
                    waits = list(si.on_wait)
                    for w in waits[:-1]:
                        nop = mybir.InstNoOp(
                            name=nc.get_next_instruction_name(),
                            engine=inst.engine,
                            bass_nofuse=True,
                            sync_info=mybir.SyncInfo(on_wait=[w], on_update=[]),
                        )
                        nc.register_instruction(nop)
                        out.append(nop)
                    inst.sync_info = mybir.SyncInfo(
                        on_wait=[waits[-1]], on_update=list(si.on_update)
                    )
                    changed = True
                out.append(inst)
            if changed:
                bb.instructions = out


def _make_amats(weight):
    """A[o,t][r*8+p', s*8+P] = W[16o + r - s, 8t + p' - P]."""
    A = np.zeros((2, 3, 128, 128), np.float32)
    r, p, s, P = np.indices((S, PHI, S, PHI))
    for o in range(2):
        for t in range(3):
            di = S * o + r - s
            dj = PHI * t + p - P
            valid = (di >= 0) & (di < KH) & (dj >= 0) & (dj < KW)
            A[o, t][(r * PHI + p)[valid], (s * PHI + P)[valid]] = weight[
                di[valid], dj[valid]
            ]
    # -> [128 k, 6*128] partition-major, stationary (o,t) at cols 128*(3o+t)
    return np.ascontiguousarray(A.reshape(6, 128, 128).transpose(1, 0, 2).reshape(128, 6 * 128))


def _build_program(bias_val):
    import concourse.bass as bass
    import concourse.mybir as mybir
    import concourse.tile as tile

    _patch_drain()
    bf16 = mybir.dt.bfloat16
    f32 = mybir.dt.float32

    nc = bass.Bass()
    xph = nc.declare_dram_parameter("xph", [128, NG * XQ], bf16, isOutput=False)
    amat = nc.declare_dram_parameter("amat", [128, 6 * 128], bf16, isOutput=False)
    outp = nc.declare_dram_parameter("outp", [128, NBLK * NQ], bf16, isOutput=True)

    with tile.TileContext(nc) as tc:
        with (
            tc.tile_pool(name="const", bufs=1) as constp,
            tc.tile_pool(name="xp", bufs=12) as xp,
            tc.tile_pool(name="psum", bufs=6, space="PSUM") as psp,
            tc.tile_pool(name="op", bufs=6) as op,
        ):
            # All 6 stationary matrices in ONE transfer on the Activation
            # ring: each dma_start's completion semaphore fires ~2us after
            # its data lands (write-receipt round trip), so six serialized
            # loads would gate the first six matmuls one by one.
            atall = constp.tile([128, 6 * 128], bf16, tag="atall")
            nc.scalar.dma_start(out=atall[:, :], in_=amat[:, :])
            ats = [atall[:, 128 * i : 128 * (i + 1)] for i in range(6)]

            # HAM warmup: the PE clock-gate needs ~3.4us of sustained busy
            # to lift 1.2 -> 2.4 GHz, and the first real matmul can't start
            # until the first X/A DMAs land (~8us in).  Keep the PE busy
            # through that window with dependency-free matmuls on a zeroed
            # scratch tile so the real stream runs warm from its first MM.
            wsrc = constp.tile([128, 128], bf16, tag="wsrc")
            nc.vector.memset(wsrc[:, :], 0.0)
            wps = psp.tile([128, 128], f32, tag="wps", bufs=1)
            NWARM = 24
            for w in range(NWARM):
                nc.tensor.matmul(
                    wps[:, :],
                    wsrc[:, :],
                    wsrc[:, :],
                    start=(w == 0),
                    stop=(w == NWARM - 1),
                )

            # Input row-groups are loaded two per dma_start: the kernel-tail
            # drain serializes one ~110ns semaphore wait per DMA per engine,
            # so halving the DMA count directly shortens the exec window.
            xtiles = {}

            def get_x(g):
                if g not in xtiles:
                    g0 = g if (g % 2 == 0) else g - 1
                    ng = min(2, NG - g0)
                    t = xp.tile([128, ng * XQ], bf16, tag="xt")
                    nc.sync.dma_start(
                        out=t[:, :], in_=xph[:, XQ * g0 : XQ * (g0 + ng)]
                    )
                    for i in range(ng):
                        xtiles[g0 + i] = t[:, XQ * i : XQ * (i + 1)]
                return xtiles[g]

            def do_block(b, q0, qn, ot):
                """Output block b, macro-columns [q0, q0+qn), evac into ot."""
                x0 = get_x(b)
                x1 = get_x(b + 1)
                ps = psp.tile([128, qn], f32, tag="ps")
                k = 0
                for o, xt in ((0, x0), (1, x1)):
                    for t in range(3):
                        nc.tensor.matmul(
                            ps[:, :],
                            ats[3 * o + t][:, :],
                            xt[:, q0 + t : q0 + t + qn],
                            start=(k == 0),
                            stop=(k == 5),
                        )
                        k += 1
                nc.vector.tensor_scalar_add(ot[:, :], ps[:, :], bias_val)

            # Output stores batched 4 blocks per dma_start (fewer DMAs ->
            # shorter drain), tapering to small stores at the end so the
            # final store's SDMA + receipt chain trails a small tile.
            for b0 in range(0, 28, 4):
                ob = op.tile([128, 4 * NQ], bf16, tag="ob", bufs=3)
                for i in range(4):
                    do_block(b0 + i, 0, NQ, ob[:, NQ * i : NQ * (i + 1)])
                nc.scalar.dma_start(
                    out=outp[:, NQ * b0 : NQ * (b0 + 4)], in_=ob[:, :]
                )
            ob = op.tile([128, 2 * NQ], bf16, tag="ob2", bufs=1)
            do_block(28, 0, NQ, ob[:, :NQ])
            do_block(29, 0, NQ, ob[:, NQ:])
            nc.scalar.dma_start(out=outp[:, NQ * 28 : NQ * 30], in_=ob[:, :])
            ob = op.tile([128, NQ], bf16, tag="ob1", bufs=1)
            do_block(30, 0, NQ, ob[:, :])
            nc.scalar.dma_start(out=outp[:, NQ * 30 : NQ * 31], in_=ob[:, :])
            # Final block in two half-width passes: the first half's store
            # overlaps the second half's matmuls.
            for h in range(2):
                oh = op.tile([128, NQ // 2], bf16, tag=f"oh{h}", bufs=1)
                do_block(NBLK - 1, h * (NQ // 2), NQ // 2, oh[:, :])
                nc.scalar.dma_start(
                    out=outp[
                        :, NQ * 31 + h * (NQ // 2) : NQ * 31 + (h + 1) * (NQ // 2)
                    ],
                    in_=oh[:, :],
                )

    _split_multi_waits(nc)
    return nc


def kernel(x, weight, bias):
    global LAST_RESULT
    import ml_dtypes
    from concourse.bass_utils import run_bass_kernel_spmd

    bf16 = ml_dtypes.bfloat16
    x = np.ascontiguousarray(np.asarray(x, dtype=np.float32))
    weight = np.asarray(weight, dtype=np.float32)
    bias = np.asarray(bias, dtype=np.float32)

    # Zero-padded image (PAD top/left, zeros beyond to fixed slab size),
    # then phase-major: XPH[g][r*8+p', Q] = xpad[16g+r, 8Q+p'].
    xbig = np.zeros((XROWS, XCOLS), bf16)
    xbig[PAD : PAD + H, PAD : PAD + W] = x.astype(bf16)
    XPH = np.ascontiguousarray(
        xbig.reshape(XROWS // 16, 16, XQ, PHI)
        .transpose(0, 1, 3, 2)
        .reshape(XROWS // 16, 128, XQ)
    )
    amats = _make_amats(weight).astype(bf16)

    nc = _build_program(float(bias[0]))
    in_maps = []
    for c in range(NCORES):
        g0 = (CROWS * c) // S
        in_maps.append(
            {
                "xph": np.ascontiguousarray(
                    XPH[g0 : g0 + NG].transpose(1, 0, 2).reshape(128, NG * XQ)
                ),
                "amat": amats,
            }
        )
    res = run_bass_kernel_spmd(
        nc,
        in_maps,
        list(range(NCORES)),
        trace=bool(os.environ.get("CONV_TRACE")),
    )
    LAST_RESULT = res

    full = np.empty((NCORES * CROWS, PHI * NQ), np.float32)
    for c in range(NCORES):
        oc = res.results[c]["outp"].astype(np.float32)
        # oc[(s,P), 512b+Q] -> rows 512c+16b+s, cols 8Q+P
        full[CROWS * c : CROWS * (c + 1)] = (
            oc.reshape(S, PHI, NBLK, NQ).transpose(2, 0, 3, 1).reshape(CROWS, PHI * NQ)
        )
    return np.ascontiguousarray(full[:OUT, :OUT]).astype(np.float32)


# revision 18
# speedup vs baseline: 1.0313x; 1.0313x over previous
"""Trainium2 Bass kernel: single-channel 15x15 cross-correlation (pad=1,
stride=1) of a 4096x4096 fp32 image, + scalar bias.

Strategy: phase-packed matmuls
------------------------------
Columns are split into PHI=8 phases (j = 8Q + P).  An output block of
S=16 rows x 4096 cols is one PSUM tile [128, 512]: partition m=(s,P)
(16 rows x 8 phases), free n=Q (512 macro-columns).  The contraction
dim packs k=(r,p') = 16 input rows x 8 phases = 128 full partitions.
Six stationary matrices A[o,t][(r,p'),(s,P)] = W[16o+r-s, 8t+p'-P]
(o: input row-group offset 0/1, t: macro-column shift 0/1/2) turn the
whole conv into 6 accumulating matmuls of free size 512 per block:

    out_block += A[o,t]^T @ X[b+o][:, t:t+512]

where X[g] is input row-group g in phase-major layout.  Useful MAC
density is 29.3% of the 128x128 array (vs 10.4% for a 15-tap banded
formulation): 6 x 512 PE cycles per 16x4096 output block.

All operands are bf16 (PSUM accumulates fp32; rel err ~3e-3), which
halves DMA bytes.  Host prepares phase-major inputs / unpacks
phase-major outputs, so every DMA is >=1KB-contiguous per partition.
Each core owns 512 output rows = 32 blocks; 33 input row-group tiles
are loaded once each and reused by adjacent blocks (o=0/1).

Timing notes (measured): warm matmul N=512 spacing is 216ns (LDWEIGHTS
hides under FWL); a dma_start's completion semaphore fires ~2us after
the data lands, so the 6 stationaries ride ONE transfer and the first
two row-groups ride one transfer; dependency-free warmup matmuls keep
the PE busy from the post-preamble barrier (~7.3us) until real work so
the HAM clock-gate (1.2 -> 2.4 GHz after ~3.4us busy) lifts early; the
final block stores in two half-tiles so the tail (store + receipt +
fixed ~10us drain epilogue) trails minimal work.  147.8us -> ~57.5us.
"""

import os

import numpy as np

KH = KW = 15
PAD = 1
H = W = 4096
OUT = H + 2 * PAD - KH + 1  # 4084
NCORES = 8
S = 16  # output rows per block
PHI = 8  # column phases
NQ = 512  # macro-columns per PSUM tile (8*512 = 4096 output cols)
XQ = 516  # macro-columns per input tile (>= NQ + 2 shifts, 4B-aligned)
NBLK = 32  # blocks per core: 32*16 = 512 output rows
NG = NBLK + 1  # input row-group tiles per core
CROWS = NBLK * S  # 512 output rows per core
XROWS = 257 * 16  # 4112 padded input rows (257 groups of 16)
XCOLS = PHI * XQ  # 4128 padded input cols

LAST_RESULT = None  # BassKernelResults of the most recent run (for test.py)


def _patch_drain():
    """walrus's CTRL_NO instruction struct holds very few semaphore waits;
    Tile's kernel-tail drain aggregates one wait per logical processor and
    overflows it.  Spread the waits across 1-wait-per-nop SP instructions."""
    import concourse.mybir as mybir
    import concourse.tile as tile
    from concourse.vector_clock import ScopedClock

    def _split_drain_and_barrier(self, tick_clock, wait_clock):
        nc = self.nc
        probe = nc.sync.nop(nofuse=True)
        wait_clock.add_sem_waits(
            probe.ins, ScopedClock({None: tick_clock.global_clock})
        )
        si = probe.ins.sync_info
        if si is not None and len(si.on_wait) > 1:
            waits = list(si.on_wait)
            probe.ins.sync_info = mybir.SyncInfo(
                on_wait=waits[:1], on_update=list(si.on_update)
            )
            for w in waits[1:]:
                extra = nc.sync.nop(nofuse=True)
                extra.ins.sync_info = mybir.SyncInfo(on_wait=[w], on_update=[])
        nc.sync.drain()
        # The stock exit path does barrier -> semaphore cleanup -> barrier
        # (~8us).  This NEFF executes once per load, so leftover semaphore
        # values don't matter: skip the cleanup, keep only the drain (which
        # carries the waits that guarantee all DMAs have landed).
        assert self.sems is not None
        popped = nc._tile_sem_poison_stack.pop()
        assert popped is self._sem_poison

    tile.TileContext._drain_and_barrier = _split_drain_and_barrier


def _split_multi_waits(nc):
    """This compiler's TPB instruction structs hold only one sync-wait slot
    (walrus setupSyncWait rejects more).  Tile sometimes assigns 2+ waits
    (DMA completion + slot release) to one instruction; split the excess onto
    same-engine nops inserted immediately before it."""
    import concourse.mybir as mybir

    for fn in nc.m.functions:
        for bb in fn.blocks:
            insts = list(bb.instructions)
            out = []
            changed = False
            for inst in insts:
                si = inst.sync_info
                if (
                    not isinstance(inst, mybir.InstNoOp)
                    and si is not None
                    and len(si.on_wait) > 1
                ):
                    waits = list(si.on_wait)
                    for w in waits[:-1]:
                        nop = mybir.InstNoOp(
                            name=nc.get_next_instruction_name(),
                            engine=inst.engine,
                            bass_nofuse=True,
                            sync_info=mybir.SyncInfo(on_wait=[w], on_update=[]),
                        )
                        nc.register_instruction(nop)
                        out.append(nop)
                    inst.sync_info = mybir.SyncInfo(
                        on_wait=[waits[-1]], on_update=list(si.on_update)
                    )
                    changed = True
                out.append(inst)
            if changed:
                bb.instructions = out


def _make_amats(weight):
    """A[o,t][r*8+p', s*8+P] = W[16o + r - s, 8t + p' - P]."""
    A = np.zeros((2, 3, 128, 128), np.float32)
    r, p, s, P = np.indices((S, PHI, S, PHI))
    for o in range(2):
        for t in range(3):
            di = S * o + r - s
            dj = PHI * t + p - P
            valid = (di >= 0) & (di < KH) & (dj >= 0) & (dj < KW)
            A[o, t][(r * PHI + p)[valid], (s * PHI + P)[valid]] = weight[
                di[valid], dj[valid]
            ]
    # -> [128 k, 6*128] partition-major, stationary (o,t) at cols 128*(3o+t)
    return np.ascontiguousarray(A.reshape(6, 128, 128).transpose(1, 0, 2).reshape(128, 6 * 128))


def _build_program(bias_val):
    import concourse.bass as bass
    import concourse.mybir as mybir
    import concourse.tile as tile

    _patch_drain()
    bf16 = mybir.dt.bfloat16
    f32 = mybir.dt.float32

    nc = bass.Bass()
    xph = nc.declare_dram_parameter("xph", [128, NG * XQ], bf16, isOutput=False)
    amat = nc.declare_dram_parameter("amat", [128, 6 * 128], bf16, isOutput=False)
    outp = nc.declare_dram_parameter("outp", [128, NBLK * NQ], bf16, isOutput=True)

    with tile.TileContext(nc) as tc:
        with (
            tc.tile_pool(name="const", bufs=1) as constp,
            tc.tile_pool(name="xp", bufs=12) as xp,
            tc.tile_pool(name="psum", bufs=6, space="PSUM") as psp,
            tc.tile_pool(name="op", bufs=6) as op,
        ):
            # All 6 stationary matrices in ONE transfer on the Activation
            # ring: each dma_start's completion semaphore fires ~2us after
            # its data lands (write-receipt round trip), so six serialized
            # loads would gate the first six matmuls one by one.
            atall = constp.tile([128, 6 * 128], bf16, tag="atall")
            nc.scalar.dma_start(out=atall[:, :], in_=amat[:, :])
            ats = [atall[:, 128 * i : 128 * (i + 1)] for i in range(6)]

            # HAM warmup: the PE clock-gate needs ~3.4us of sustained busy
            # to lift 1.2 -> 2.4 GHz, and the first real matmul can't start
            # until the first X/A DMAs land (~8us in).  Keep the PE busy
            # through that window with dependency-free matmuls on a zeroed
            # scratch tile so the real stream runs warm from its first MM.
            wsrc = constp.tile([128, 128], bf16, tag="wsrc")
            nc.vector.memset(wsrc[:, :], 0.0)
            wps = psp.tile([128, 128], f32, tag="wps", bufs=1)
            NWARM = 24
            for w in range(NWARM):
                nc.tensor.matmul(
                    wps[:, :],
                    wsrc[:, :],
                    wsrc[:, :],
                    start=(w == 0),
                    stop=(w == NWARM - 1),
                )

            xtiles = {}
            # First transfer covers groups 0+1 together so block 0's o=1
            # matmuls aren't gated on a second DMA's +2us receipt.
            x01 = xp.tile([128, 2 * XQ], bf16, tag="xt01", bufs=1)
            nc.sync.dma_start(out=x01[:, :], in_=xph[:, : 2 * XQ])
            xtiles[0] = x01[:, :XQ]
            xtiles[1] = x01[:, XQ:]

            def get_x(g):
                if g not in xtiles:
                    t = xp.tile([128, XQ], bf16, tag="xt")
                    nc.sync.dma_start(out=t[:, :], in_=xph[:, XQ * g : XQ * (g + 1)])
                    xtiles[g] = t
                return xtiles[g]

            def do_block(b, q0, qn, ot):
                """Output block b, macro-columns [q0, q0+qn), evac into ot."""
                x0 = get_x(b)
                x1 = get_x(b + 1)
                ps = psp.tile([128, qn], f32, tag="ps")
                k = 0
                for o, xt in ((0, x0), (1, x1)):
                    for t in range(3):
                        nc.tensor.matmul(
                            ps[:, :],
                            ats[3 * o + t][:, :],
                            xt[:, q0 + t : q0 + t + qn],
                            start=(k == 0),
                            stop=(k == 5),
                        )
                        k += 1
                nc.vector.tensor_scalar_add(ot[:, :], ps[:, :], bias_val)

            for b in range(NBLK - 1):
                ot = op.tile([128, NQ], bf16, tag="ot")
                do_block(b, 0, NQ, ot[:, :])
                nc.scalar.dma_start(out=outp[:, NQ * b : NQ * (b + 1)], in_=ot[:, :])
            # Final block in two half-width passes so the last evac + store
            # chain (DVE + DMA + receipt) trails a half-size tile, and the
            # first half's store overlaps the second half's matmuls.
            for h in range(2):
                oh = op.tile([128, NQ // 2], bf16, tag=f"oh{h}", bufs=1)
                do_block(NBLK - 1, h * (NQ // 2), NQ // 2, oh[:, :])
                nc.scalar.dma_start(
                    out=outp[
                        :, NQ * 31 + h * (NQ // 2) : NQ * 31 + (h + 1) * (NQ // 2)
                    ],
                    in_=oh[:, :],
                )

    _split_multi_waits(nc)
    return nc


def kernel(x, weight, bias):
    global LAST_RESULT
    import ml_dtypes
    from concourse.bass_utils import run_bass_kernel_spmd

    bf16 = ml_dtypes.bfloat16
    x = np.ascontiguousarray(np.asarray(x, dtype=np.float32))
    weight = np.asarray(weight, dtype=np.float32)
    bias = np.asarray(bias, dtype=np.float32)

    # Zero-padded image (PAD top/left, zeros beyond to fixed slab size),
    # then phase-major: XPH[g][r*8+p', Q] = xpad[16g+r, 8Q+p'].
    xbig = np.zeros((XROWS, XCOLS), bf16)
    xbig[PAD : PAD + H, PAD : PAD + W] = x.astype(bf16)
    XPH = np.ascontiguousarray(
        xbig.reshape(XROWS // 16, 16, XQ, PHI)
        .transpose(0, 1, 3, 2)
        .reshape(XROWS // 16, 128, XQ)
    )
    amats = _make_amats(weight).astype(bf16)

    nc = _build_program(float(bias[0]))
    in_maps = []
    for c in range(NCORES):
        g0 = (CROWS * c) // S
        in_maps.append(
            {
                "xph": np.ascontiguousarray(
                    XPH[g0 : g0 + NG].transpose(1, 0, 2).reshape(128, NG * XQ)
                ),
                "amat": amats,
            }
        )
    res = run_bass_kernel_spmd(
        nc,
        in_maps,
        list(range(NCORES)),
        trace=bool(os.environ.get("CONV_TRACE")),
    )
    LAST_RESULT = res

    full = np.empty((NCORES * CROWS, PHI * NQ), np.float32)
    for c in range(NCORES):
        oc = res.results[c]["outp"].astype(np.float32)
        # oc[(s,P), 512b+Q] -> rows 512c+16b+s, cols 8Q+P
        full[CROWS * c : CROWS * (c + 1)] = (
            oc.reshape(S, PHI, NBLK, NQ).transpose(2, 0, 3, 1).reshape(CROWS, PHI * NQ)
        )
    return np.ascontiguousarray(full[:OUT, :OUT]).astype(np.float32)
